# revision 9
# baseline (speedup 1.0000x reference)
"""3x3 median filter (reflect padding) on Trainium2, data-parallel over batch.

Input:  image [16, 3, 512, 512] f32
Output: same shape; out[b,c,y,x] = median of the 3x3 window around (y,x),
        reflect padding.

Sharding: batch dim split across 8 NeuronCores (2 images per core), SPMD.

All VectorE TENSOR_TENSOR ops run in bf16 with dense step-1, 4-byte-aligned
access patterns so every one hits the DVE 2x_1P perf mode (2 elem/cycle/lane)
instead of the 1x floor fp32 TT is stuck at. bf16 keeps rel-err ~2^-8
(<< 2e-2 tolerance) with a full 8-bit exponent (no subnormal blowup near
the harness' 1e-6 denom floor).

Host prep: per-core input is transposed+padded to [BPC, H+2, C, W+2] bf16
with BOTH the vertical and horizontal reflect borders pre-staged. The
horizontal pad removes all boundary-column special cases; every op is a
uniform dense sweep over flattened [C, W+2] (the 2 pad cols per channel
compute garbage that is simply not stored).

Per-core compute, per superstep (2 row-tiles of 128 rows fused into one
free-dim span to halve instruction count and amortize the ~150-cycle DVE
instruction init):
  1. one contiguous 3-row DMA per tile; vertical sort3 -> lo/md/hi
     (6 TT, partially in-place)
  2. ScalarE makes the ONE odd-shift copy s1[k][x] = lmh[k][x+1] (ScalarE
     is 1x regardless, so misalignment is free there; it runs in the
     pipeline shadow of the next tile's vertical sort).
  3. pair stage:   Pmax_lo,Pmax_md = max(lmh[0:2], s1[0:2])
                   Pmin_md,Pmin_hi = min(lmh[1:3], s1[1:3])   (2 stacked TT)
  4. combine with the +2 (even, aligned) shift:
       A  = max(Pmax_lo, lo+2)          t3 = min(Pmax_md, md+2)
       Cc = min(Pmin_hi, hi+2)          B  = max(Pmin_md, t3) (3 TT)
  5. median = med3(A, B, Cc)                                  (4 TT)
Software pipeline: stage_a(t+1) [DMA+vertical+s1] is emitted before
stage_b(t) [pairs..median] so the ScalarE copy never stalls VectorE.
"""

import sys

sys.path.insert(0, "/opt/trn_rl_repo")

import numpy as np

_COMPILED = {}

B, C, H, W = 16, 3, 512, 512
NCORES = 8
BPC = B // NCORES  # batches per core
RT = 128           # rows per tile (partition dim)
TPS = 2            # row-tiles fused per superstep
NSS = H // (RT * TPS)  # supersteps per batch
HP = H + 2         # padded rows on device
WP = W + 2         # padded cols on device
SRP = C * WP       # padded row stride (elements), 1542
SB = HP * SRP      # batch stride (input)
SR = C * W         # output row stride
SBO = H * SR       # batch stride (output)
FDS = SRP              # one lmh slice, flattened [C, WP]
FDC = 2 * WP + W       # combine width 1540: covers flat c*WP+x, x<W
R3T = 3 * FDS          # rows3 / lmh tile stride (elements)
ST = 4 * FDS           # S tile stride
TCT = 2 * FDC          # TC tile stride
NS1 = TPS * R3T        # full s1 flat span (9252)


def _legalize_waits(nc, mybir):
    """Hoist excess sync-waits into a preceding same-engine EventSemaphore.
    The TRN2 ISA allows 1 sync-wait on compute instructions (EventSemaphore
    allows more) but Tile's scheduler can emit more; a wait-only instruction
    earlier in the same engine's program order is semantically identical."""
    limits = {"InstEventSemaphore": 2}
    n_hoisted = 0
    for f in nc.m.functions:
        for bb in f.blocks:
            il = bb.instructions
            idx = 0
            while idx < len(il):
                i = il[idx]
                si = i.sync_info
                lim = limits.get(type(i).__name__, 1)
                if si is not None and si.on_wait and len(si.on_wait) > lim:
                    waits = list(si.on_wait)
                    keep, excess = waits[:lim], waits[lim:]
                    hoists = []
                    for j in range(0, len(excess), 2):
                        h = mybir.InstEventSemaphore(
                            name=f"hoistw_{n_hoisted}", ins=[], outs=[])
                        n_hoisted += 1
                        h.engine = i.engine
                        h.sync_info = mybir.SyncInfo(
                            on_wait=excess[j:j + 2], on_update=[])
                        hoists.append(h)
                    i.sync_info = mybir.SyncInfo(
                        on_wait=keep, on_update=si.on_update)
                    for k, h in enumerate(hoists):
                        il.insert(idx + k, h)
                    idx += len(hoists)
                idx += 1
    return n_hoisted


def _build_nc():
    from concourse import bass
    import concourse.mybir as mybir
    from concourse.tile import TileContext

    bf16 = mybir.dt.bfloat16
    MIN = mybir.AluOpType.min
    MAX = mybir.AluOpType.max
    AP = bass.AP

    nc = bass.Bass()
    img = nc.dram_tensor("image", [BPC, HP, C, WP], bf16, kind="ExternalInput")
    out = nc.dram_tensor("out", [BPC, H, C, W], bf16, kind="ExternalOutput")

    def fv(tile_ap, off, dims):
        """Free-dim view of an SBUF tile: keep partition dim, replace free
        dims with `dims`, shift base by `off` elements."""
        return AP(tile_ap.tensor, tile_ap.offset + off,
                  [list(tile_ap.ap[0])] + [list(d) for d in dims])

    def stage_a(nc, pool, g, s):
        """2x [DMA 3 contiguous rows] + vertical sort3 + ScalarE shift copy.

        Returns (lmh, s1). Free-dim layouts per partition (elements):
          rows3 [TPS, 3, FDS]  lmh [TPS, 3, FDS]  s1 [TPS, 3, FDS]
        """
        rows3 = pool.tile([RT, TPS, 3, C, WP], bf16, tag="rows3")
        r3 = rows3[:]
        for t in range(TPS):
            r0 = (s * TPS + t) * RT
            nc.sync.dma_start(
                out=fv(r3, t * R3T, [[1, R3T]]),
                in_=AP(img, g * SB + r0 * SRP, [[SRP, RT], [1, R3T]]))

        # vertical sort3 over both tiles at once; window rows for output
        # row r are padded rows r..r+2 = rows3[t, 0..2]
        lmh = pool.tile([RT, TPS, 3, C, WP], bf16, tag="lmh")
        m = pool.tile([RT, TPS, C, WP], bf16, tag="m", bufs=1)
        lv = lmh[:]
        TT = nc.vector.tensor_tensor
        third = fv(r3, 0 * FDS, [[R3T, TPS], [1, FDS]])
        pa = fv(r3, 1 * FDS, [[R3T, TPS], [1, FDS]])
        pb = fv(r3, 2 * FDS, [[R3T, TPS], [1, FDS]])
        lo = fv(lv, 0 * FDS, [[R3T, TPS], [1, FDS]])
        md = fv(lv, 1 * FDS, [[R3T, TPS], [1, FDS]])
        hi = fv(lv, 2 * FDS, [[R3T, TPS], [1, FDS]])
        mm = m[:]
        TT(lo, pa, pb, MIN)          # t1 -> lo slot
        TT(hi, pa, pb, MAX)          # t2 -> hi slot
        TT(mm, hi, third, MIN)       # m = min(t2, third)
        TT(hi, hi, third, MAX)       # hi = max(t2, third)   (in place)
        TT(md, lo, mm, MAX)          # md = max(t1, m)  (before lo overwrite)
        TT(lo, lo, mm, MIN)          # lo = min(t1, m)       (in place)

        # the single odd shift, on ScalarE: s1[k][x] = lmh[k][x+1] flat
        s1 = pool.tile([RT, TPS, 3, C, WP], bf16, tag="s1")
        nc.scalar.copy(fv(s1[:], 0, [[1, NS1 - 1]]),
                       fv(lv, 1, [[1, NS1 - 1]]))
        # last element feeds only a discarded pad column; init to keep
        # uninitialized-read checks quiet
        nc.vector.memset(fv(s1[:], NS1 - 1, [[1, 1]]), 0.0)
        return lmh, s1

    def stage_b(nc, pool, g, s, lmh, s1):
        """Pairs + combine + med3 + 2x DMA out, consuming stage_a tiles."""
        lv, sv = lmh[:], s1[:]
        TT = nc.vector.tensor_tensor
        # S: 0=Pmax_lo 1=Pmax_md 2=Pmin_md 3=Pmin_hi (per tile)
        S = pool.tile([RT, TPS, 4, C, WP], bf16, tag="S", bufs=1)
        Sv = S[:]
        d2 = [[R3T, TPS], [FDS, 2], [1, FDS]]
        TT(fv(Sv, 0, [[ST, TPS], [FDS, 2], [1, FDS]]),
           fv(lv, 0, d2), fv(sv, 0, d2), MAX)
        TT(fv(Sv, 2 * FDS, [[ST, TPS], [FDS, 2], [1, FDS]]),
           fv(lv, FDS, d2), fv(sv, FDS, d2), MIN)

        # combine with even +2 shifts:
        # TC[t][0]=t3=min(Pmax_md, md+2); TC[t][1]=Cc=min(Pmin_hi, hi+2)
        TC = pool.tile([RT, TPS, 2, FDC], bf16, tag="TC", bufs=1)
        A = pool.tile([RT, TPS, FDC], bf16, tag="A", bufs=1)
        Bt = pool.tile([RT, TPS, FDC], bf16, tag="Bt", bufs=1)
        TT(TC[:],
           fv(Sv, FDS, [[ST, TPS], [2 * FDS, 2], [1, FDC]]),
           fv(lv, FDS + 2, [[R3T, TPS], [FDS, 2], [1, FDC]]),
           MIN)
        TT(A[:],
           fv(Sv, 0, [[ST, TPS], [1, FDC]]),
           fv(lv, 2, [[R3T, TPS], [1, FDC]]),
           MAX)
        TT(Bt[:],
           fv(Sv, 2 * FDS, [[ST, TPS], [1, FDC]]),
           fv(TC[:], 0, [[TCT, TPS], [1, FDC]]),
           MAX)

        # final med3(A, B, Cc), both tiles per instruction (A in-place)
        f1 = pool.tile([RT, TPS, FDC], bf16, tag="f1", bufs=1)
        res = pool.tile([RT, TPS, FDC], bf16, tag="res")
        dc = [[FDC, TPS], [1, FDC]]
        Av = fv(A[:], 0, dc)
        Bv = fv(Bt[:], 0, dc)
        Fv = fv(f1[:], 0, dc)
        Cv = fv(TC[:], FDC, [[TCT, TPS], [1, FDC]])
        TT(Fv, Av, Bv, MIN)
        TT(Av, Av, Bv, MAX)
        TT(Av, Av, Cv, MIN)
        TT(fv(res[:], 0, dc), Fv, Av, MAX)

        # store: out[r, c, x] = res[t][c*WP + x]
        for t in range(TPS):
            r0 = (s * TPS + t) * RT
            nc.sync.dma_start(
                out=AP(out, g * SBO + r0 * SR, [[SR, RT], [W, C], [1, W]]),
                in_=fv(res[:], t * FDC, [[WP, C], [1, W]]))

    with TileContext(nc) as tc:
        with tc.tile_pool(name="p", bufs=2) as pool:
            # Software pipeline: emit stage_a(t+1) before stage_b(t) so the
            # ScalarE shift-copy of superstep t overlaps VectorE's vertical
            # sort of superstep t+1 instead of stalling the pair stage.
            steps = [(g, s) for g in range(BPC) for s in range(NSS)]
            prev = None
            for (g, s) in steps:
                cur = stage_a(nc, pool, g, s)
                if prev is not None:
                    stage_b(nc, pool, *prev[0], *prev[1])
                prev = ((g, s), cur)
            stage_b(nc, pool, *prev[0], *prev[1])

    _legalize_waits(nc, mybir)
    return nc


def _stage_input(img_k: np.ndarray) -> np.ndarray:
    """[BPC, C, H, W] f32 -> reflect-padded transposed [BPC, HP, C, WP] bf16."""
    import ml_dtypes

    t = img_k.transpose(0, 2, 1, 3).astype(ml_dtypes.bfloat16)  # [BPC,H,C,W]
    p = np.empty((BPC, HP, C, WP), dtype=ml_dtypes.bfloat16)
    p[:, 1:H + 1, :, 1:W + 1] = t
    p[:, 1:H + 1, :, 0] = t[:, :, :, 1]          # col -1 = col 1
    p[:, 1:H + 1, :, W + 1] = t[:, :, :, W - 2]  # col W  = col W-2
    p[:, 0] = p[:, 2]          # row -1 = row 1
    p[:, H + 1] = p[:, H - 1]  # row H  = row H-2
    return p


def kernel(image: np.ndarray) -> np.ndarray:
    from concourse.bass_utils import run_bass_kernel_spmd

    image = np.asarray(image, dtype=np.float32)
    if "nc" not in _COMPILED:
        _COMPILED["nc"] = _build_nc()
    nc = _COMPILED["nc"]

    in_maps = [{"image": _stage_input(image[k * BPC:(k + 1) * BPC])}
               for k in range(NCORES)]
    try:
        res = run_bass_kernel_spmd(nc, in_maps, core_ids=list(range(NCORES)))
    except Exception:
        # transient accelerator errors (e.g. NRT_EXEC_UNIT_UNRECOVERABLE)
        # have been observed to clear on retry
        res = run_bass_kernel_spmd(nc, in_maps, core_ids=list(range(NCORES)))
    return np.concatenate(
        [np.asarray(res.results[k]["out"]).astype(np.float32)
         .transpose(0, 2, 1, 3) for k in range(NCORES)],
        axis=0)
